# revision 42
# baseline (speedup 1.0000x reference)
"""Multi-head causal self-attention (B=4, T=2048, C=1024, H=16) on 8 TRN2 cores.

Sharding: core c handles batch b = c//2 and head-group hg = c%2 (8 heads as 4
pairs): data parallel over B, tensor parallel over H. Pipelined schedule: qkv
projection runs in four 512-column t-chunks; attention round qc starts as soon
as chunk qc is done, so the softmax exp (ScalarE) overlaps the remaining
projection matmuls. Scores use row-tiled K=64 matmul pairs (head A on
partitions 0:64, head B on 64:128) sharing one 512-column q stream — both run
concurrently in the PE array. Exp is batched: one ACTIVATE covers a 4-bank
[128, 2048] PSUM tile (2 key-blocks x 2 heads). Softmax denominator comes from
a ones-row fused into the attn@V lhsT; normalization uses
reciprocal_approx_fast + per-pair K=2 broadcast matmuls.
"""

from collections import deque
from contextlib import ExitStack

import ml_dtypes
import numpy as np

import concourse.bass as bass
import concourse.bacc as bacc
import concourse.mybir as mybir
import concourse.tile as tile
from concourse.bass_utils import run_bass_kernel_spmd
from concourse.masks import make_upper_triangular

B, T, C, H, HS = 4, 2048, 1024, 16, 64
P = 128
NKB = T // P            # key blocks of 128
SCALE = HS ** -0.5

F32 = mybir.dt.float32
F32R = mybir.dt.float32r
BF16 = mybir.dt.bfloat16
Exp = mybir.ActivationFunctionType.Exp


def build_kernel():
    nc = bacc.Bacc("TRN2", target_bir_lowering=False)

    # all inputs pre-swizzled on host into SBUF layout: straight row-contiguous
    # copies with large DMA descriptors (fast per-queue transfer)
    xt_d = nc.dram_tensor("xt", (P, 4 * 4096), BF16, kind="ExternalInput")
    wqk_d = nc.dram_tensor("wqk", (P, 8 * 1024), BF16, kind="ExternalInput")
    bqk_d = nc.dram_tensor("bqk", (8 * P,), F32, kind="ExternalInput")
    wv_d = nc.dram_tensor("wv", (P, 8 * 512), BF16, kind="ExternalInput")
    bv_d = nc.dram_tensor("bv", (1, 512), F32R, kind="ExternalInput")
    wproj_d = nc.dram_tensor("wproj", (P, 4 * C), BF16, kind="ExternalInput")
    y_d = nc.dram_tensor("y", (T, C), BF16, kind="ExternalOutput")

    with tile.TileContext(nc) as tc, ExitStack() as big:
        const = big.enter_context(tc.tile_pool(name="const", bufs=1))
        persist = big.enter_context(tc.tile_pool(name="persist", bufs=1))
        xtp = big.enter_context(tc.tile_pool(name="xtp", bufs=2))
        atp = big.enter_context(tc.tile_pool(name="atp", bufs=6))
        rzp = big.enter_context(tc.tile_pool(name="rzp", bufs=2))
        rbp = big.enter_context(tc.tile_pool(name="rbp", bufs=2))
        ysp = big.enter_context(tc.tile_pool(name="ysp", bufs=3))
        ps_s = big.enter_context(tc.tile_pool(name="ps_s", bufs=2, space="PSUM"))
        ps_po = big.enter_context(tc.tile_pool(name="ps_po", bufs=1, space="PSUM"))
        ps_acc = big.enter_context(tc.tile_pool(name="ps_acc", bufs=2, space="PSUM"))

        # ---------------- input DMAs first: no waits, big transfers --------
        wqk_sb = persist.tile([P, 8 * 1024], BF16, tag="wqk")
        wv_sb = persist.tile([P, 8 * 512], BF16, tag="wv")
        wpj = persist.tile([P, 4 * C], BF16, tag="wpj")
        bqk = persist.tile([P, 8], F32, tag="bqk")
        bias_v = persist.tile([P, 512], F32, tag="bias_v")
        bvr = const.tile([1, 512], F32R, tag="bvr")
        xtc = [None] * 4

        def load_chunk(tck, ways=2):
            xtc[tck] = xtp.tile([P, 8 * 512], BF16, tag="xT", name=f"xT{tck}")
            rows = P // ways
            for w in range(ways):
                nc.sync.dma_start(
                    xtc[tck][w * rows : (w + 1) * rows, :],
                    xt_d[w * rows : (w + 1) * rows, tck * 4096 : (tck + 1) * 4096],
                )

        # x chunk0 and wv first (v groups can start before wqk finishes)
        load_chunk(0, ways=4)
        for w in range(2):
            nc.sync.dma_start(
                wv_sb[w * 64 : (w + 1) * 64, :], wv_d[w * 64 : (w + 1) * 64, :]
            )
        nc.sync.dma_start(bvr[:], bv_d[:])
        for w in range(4):
            nc.sync.dma_start(
                wqk_sb[w * 32 : (w + 1) * 32, :], wqk_d[w * 32 : (w + 1) * 32, :]
            )
        nc.sync.dma_start(bqk[:], bqk_d[:].rearrange("(a p) -> p a", p=P))

        # ---------------- constants ----------------
        mask = const.tile([P, P], BF16, tag="mask")
        make_upper_triangular(nc, mask[:], val=1.0, diag=True)
        ones_f = const.tile([P, P], F32, tag="ones_f")
        nc.vector.memset(ones_f[:], 1.0)
        ones_t = const.tile([1, P], F32R, tag="ones")
        nc.vector.tensor_copy(ones_t[:], ones_f[0:1, :])
        # selA rows {32p}: cols 0:64 = 1; selB rows {32p}: cols 64:128 = 1
        ones_b = const.tile([P, P], BF16, tag="ones_b")
        nc.vector.memset(ones_b[:], 1.0)
        selA = const.tile([P, P], BF16, tag="selA")
        selB = const.tile([P, P], BF16, tag="selB")
        nc.vector.memset(selA[:], 0.0)
        nc.vector.memset(selB[:], 0.0)
        for pr in range(4):
            nc.sync.dma_start(
                selA[pr * 32 : pr * 32 + 1, 0:64], ones_b[0:1, 0:64]
            )
            nc.sync.dma_start(
                selB[pr * 32 : pr * 32 + 1, 64:P], ones_b[0:1, 0:64]
            )

        # ---------------- persistent tensors ----------------
        # qk_all: per pair p: block 2p = q (rows 0:64 head A, 64:128 head B),
        # block 2p+1 = k (same row split). [P, 8*T] bf16.
        qk_all = persist.tile([P, 8 * T], BF16, tag="qk")
        # v_all: per (pair, kb): [vA(64) | onesA(1) | vB(64) | onesB(1)] = 130
        v_all = persist.tile([P, 4 * NKB * 130], BF16, tag="v")
        va4 = v_all[:].rearrange("p (a b c) -> p a b c", a=4, b=NKB, c=130)
        nc.vector.tensor_copy(va4[:, :, :, 64:65], ones_f[:, 0 : 4 * NKB])
        nc.vector.tensor_copy(va4[:, :, :, 129:130], ones_f[:, 0 : 4 * NKB])
        # unnormalized attention output, pair-stacked transposed layout
        aoT = persist.tile([P, 4 * T], BF16, tag="aoT")
        # bias_v[128, 512] = b_v broadcast along partitions (K=1 matmul)
        pbv = ps_acc.tile([P, 512], F32, tag="acc")
        nc.tensor.matmul(pbv[:], ones_t[:], bvr[:], start=True, stop=True)
        nc.vector.tensor_copy(bias_v[:], pbv[:])

        # ---------------- work-item generators ----------------
        def qk_group(tck, chb):
            def run():
                pq = ps_acc.tile([P, 512], F32, tag="acc")
                for cb in range(8):
                    nc.tensor.matmul(
                        pq[:],
                        wqk_sb[:, cb * 1024 + chb * P : cb * 1024 + (chb + 1) * P],
                        xtc[tck][:, cb * 512 : (cb + 1) * 512],
                        start=(cb == 0),
                        stop=(cb == 7),
                    )
                p_pair, kind = chb // 2, chb % 2  # kind: 0 = q, 1 = k
                blk = 2 * p_pair + kind
                nc.vector.tensor_scalar_add(
                    qk_all[:, blk * T + tck * 512 : blk * T + (tck + 1) * 512],
                    pq[:],
                    bqk[:, chb : chb + 1],
                )
            return run

        def v_group(tck, tb):
            def run():
                kb = tck * 4 + tb
                pv = ps_acc.tile([P, 512], F32, tag="acc")
                for cb in range(8):
                    nc.tensor.matmul(
                        pv[:],
                        xtc[tck][:, cb * 512 + tb * P : cb * 512 + (tb + 1) * P],
                        wv_sb[:, cb * 512 : (cb + 1) * 512],
                        start=(cb == 0),
                        stop=(cb == 7),
                    )
                dst = bass.AP(
                    v_all[:].tensor,
                    v_all[:].offset + kb * 130,
                    [[v_all[:].ap[0][0], P], [NKB * 130, 4], [65, 2], [1, 64]],
                )
                src = bass.AP(
                    pv[:].tensor,
                    pv[:].offset,
                    [[pv[:].ap[0][0], P], [128, 4], [64, 2], [1, 64]],
                )
                bsrc = bass.AP(
                    bias_v[:].tensor,
                    bias_v[:].offset,
                    [[bias_v[:].ap[0][0], P], [128, 4], [64, 2], [1, 64]],
                )
                nc.vector.tensor_tensor(dst, src, bsrc, mybir.AluOpType.add)
            return run

        def chunk_items(tck):
            items = [qk_group(tck, chb) for chb in range(8)]
            items += [v_group(tck, tb) for tb in range(4)]
            return items

        def proj_group(qc, tb, oc):
            def run():
                py = ps_acc.tile([P, 512], F32, tag="acc")
                for pp in range(4):
                    nc.tensor.matmul(
                        py[:],
                        aoT[:, pp * T + tb * P : pp * T + (tb + 1) * P],
                        wpj[:, pp * C + oc * 512 : pp * C + (oc + 1) * 512],
                        start=(pp == 0),
                        stop=(pp == 3),
                    )
                ys = ysp.tile([P, 512], BF16, tag="ys")
                nc.vector.tensor_copy(ys[:], py[:])
                nc.sync.dma_start(
                    y_d[tb * P : (tb + 1) * P, oc * 512 : (oc + 1) * 512], ys[:]
                )
            return run

        def proj_items(qc):
            return [
                proj_group(qc, tb, oc)
                for tb in range(qc * 4, (qc + 1) * 4)
                for oc in range(2)
            ]

        def norm_pair(qc, rz, p_pair):
            # rz row 32p holds [Z_A (512) | Z_B (512)] for pair p. Two
            # accumulating K=1 broadcast matmuls -> pbt (rows 0:64 = Z_A,
            # 64:128 = Z_B), reciprocal after broadcast, one [128,512] mul.
            r = p_pair * 32
            pbt = ps_acc.tile([P, 512], F32, tag="acc")
            nc.tensor.matmul(
                pbt[:],
                selA[r : r + 1, :],
                rz[r : r + 1, 0:512],
                start=True,
                stop=False,
                tile_position=(r, 0),
            )
            nc.tensor.matmul(
                pbt[:],
                selB[r : r + 1, :],
                rz[r : r + 1, 512:1024],
                start=False,
                stop=True,
                tile_position=(r, 0),
            )
            rb = rbp.tile([P, 512], F32, tag="rb")
            nc.vector.reciprocal_approx_fast(rb[:], pbt[:])
            col = p_pair * T + qc * 512
            nc.vector.tensor_mul(
                aoT[:, col : col + 512], aoT[:, col : col + 512], rb[:]
            )

        # ---------------- attention rounds with static interleave ----------
        fill_q = deque()

        def drain(n):
            for _ in range(n):
                if fill_q:
                    fill_q.popleft()()

        def run_round(qc, rz, prereq=None, late_prereq=None):
            nkb = 4 * qc + 4
            n_groups_total = 4 * nkb
            fill_total = len(fill_q)
            done_fill = [0]
            done_groups = [0]

            def after_group():
                done_groups[0] += 1
                want = fill_total * done_groups[0] // n_groups_total
                d = want - done_fill[0]
                done_fill[0] += d
                drain(d)

            pending_norm = [None]
            for p_pair in range(4):
                if prereq is not None:
                    for it in prereq(p_pair):
                        it()
                qblk, kblk = 2 * p_pair, 2 * p_pair + 1
                po0 = ps_po.tile([65, 512], F32, tag="po0")
                po1 = ps_po.tile([65, 512], F32, tag="po1")
                po = [po0, po1]
                pend = None  # (at, kb)
                for kb in range(nkb):
                    if late_prereq is not None:
                        for it in late_prereq(p_pair, kb):
                            it()
                    qoff = max(0, kb * P - qc * 512)
                    S = ps_s.tile([P, 1024], F32, tag="S")
                    at = atp.tile([P, 1024], BF16, tag="at")
                    for hh in range(2):
                        r0 = hh * 64
                        nc.tensor.matmul(
                            S[:, hh * 512 + qoff : (hh + 1) * 512],
                            qk_all[r0 : r0 + 64, kblk * T + kb * P : kblk * T + (kb + 1) * P],
                            qk_all[r0 : r0 + 64, qblk * T + qc * 512 + qoff : qblk * T + (qc + 1) * 512],
                            start=True,
                            stop=True,
                        )
                    nc.scalar.activation(at[:], S[:], Exp, scale=SCALE)
                    if kb * P >= qc * 512:  # diagonal: zero k > q entries
                        # on GpSimd: keeps the Vector queue free for evictions
                        for hh in range(2):
                            c0 = hh * 512 + qoff
                            nc.gpsimd.tensor_tensor(
                                at[:, c0 : c0 + P], at[:, c0 : c0 + P], mask[:],
                                mybir.AluOpType.mult,
                            )
                    if pend is not None:
                        emit_attnv(qc, p_pair, po, *pend, nkb)
                    pend = (at, kb)
                    after_group()
                    if pending_norm[0] is not None:
                        # previous pair's normalization, off the critical path
                        norm_pair(qc, rz, pending_norm[0])
                        pending_norm[0] = None
                emit_attnv(qc, p_pair, po, *pend, nkb)
                # evict raw ao + Z rows, then normalize this pair inline
                col = p_pair * T + qc * 512
                nc.vector.tensor_copy(aoT[0:64, col : col + 512], po0[0:64, :])
                nc.vector.tensor_copy(aoT[64:P, col : col + 512], po1[0:64, :])
                r = p_pair * 32
                nc.vector.tensor_copy(rz[r : r + 1, 0:512], po0[64:65, :])
                nc.vector.tensor_copy(rz[r : r + 1, 512:1024], po1[64:65, :])
                pending_norm[0] = p_pair
            drain(len(fill_q))
            norm_pair(qc, rz, pending_norm[0])

        def emit_attnv(qc, p_pair, po, at, kb, nkb):
            qoff = max(0, kb * P - qc * 512)
            for hh in range(2):
                nc.tensor.matmul(
                    po[hh][:, qoff:512],
                    v_all[:, p_pair * NKB * 130 + kb * 130 + hh * 65 :
                          p_pair * NKB * 130 + kb * 130 + hh * 65 + 65],
                    at[:, hh * 512 + qoff : (hh + 1) * 512],
                    start=(kb == 0),
                    stop=(kb == nkb - 1),
                    skip_group_check=True,
                )

        # ---------------- main schedule ----------------
        # round 0 pair p only needs its own q/k groups (chb 2p, 2p+1) and the
        # first four v groups; emit the minimum up front, rest per-pair.
        for it in [v_group(0, tb) for tb in range(4)] + [qk_group(0, 0), qk_group(0, 1)]:
            it()

        def prereq0(p_pair):
            if p_pair == 0:
                return []
            return [qk_group(0, 2 * p_pair), qk_group(0, 2 * p_pair + 1)]

        # round 3's k and v chunk-3 groups are deferred INTO round 3 (they are
        # only needed from kb 12 onward) to fill its ACT-paced stalls.
        def late3(p_pair, kb):
            items = []
            if kb == 12:
                items.append(qk_group(3, 2 * p_pair + 1))
                if p_pair == 0:
                    items.append(v_group(3, 0))
            elif p_pair == 0 and kb in (13, 14, 15):
                items.append(v_group(3, kb - 12))
            return items

        rzs = [None] * 4
        for qc in range(4):
            rzs[qc] = rzp.tile([P, 1024], BF16, tag="rz", name=f"rz{qc}")
            if qc < 3:
                fill_q.append(lambda t=qc + 1: load_chunk(t))
            if qc == 0:
                def load_wpj():
                    for w in range(2):
                        nc.sync.dma_start(
                            wpj[w * 64 : (w + 1) * 64, :],
                            wproj_d[w * 64 : (w + 1) * 64, :],
                        )
                fill_q.append(load_wpj)
            if qc < 2:
                fill_q.extend(chunk_items(qc + 1))
            elif qc == 2:
                # only chunk 3's q groups (k/v deferred into round 3)
                fill_q.extend(qk_group(3, 2 * p) for p in range(4))
                fill_q.extend(proj_items(0))
                fill_q.extend(proj_items(1))
            if qc == 3:
                fill_q.extend(proj_items(2))
            run_round(
                qc,
                rzs[qc],
                prereq=prereq0 if qc == 0 else None,
                late_prereq=late3 if qc == 3 else None,
            )
        for it in proj_items(3):
            it()

    nc.compile()
    return nc


def _shard_inputs(x, W_qkv, b_qkv, W_proj):
    """Build the 8 per-core input maps."""
    in_maps = []
    for c in range(8):
        b = c // 2
        hg = c % 2
        heads = [hg * 8 + j for j in range(8)]
        qk_cols = []
        for p in range(4):
            ha, hb = heads[2 * p], heads[2 * p + 1]
            for part in range(2):  # q, k
                qk_cols.extend(range(ha * 192 + part * 64, ha * 192 + part * 64 + 64))
                qk_cols.extend(range(hb * 192 + part * 64, hb * 192 + part * 64 + 64))
        qk_cols = np.array(qk_cols)
        v_cols = []
        for p in range(4):
            ha, hb = heads[2 * p], heads[2 * p + 1]
            v_cols.extend(range(ha * 192 + 128, ha * 192 + 192))
            v_cols.extend(range(hb * 192 + 128, hb * 192 + 192))
        v_cols = np.array(v_cols)
        # pre-swizzle into the exact SBUF layouts (row-contiguous for fast DMA)
        # xt[p, tck*4096 + cb*512 + t'] = x[b][tck*512 + t', cb*128 + p]
        xt = (
            x[b].reshape(4, 512, 8, 128).transpose(3, 0, 2, 1).reshape(128, 16384)
        )
        # wqk[p, cb*1024 + ch] = W_qkv[cb*128 + p, qk_cols[ch]]
        wqk = (
            W_qkv[:, qk_cols].reshape(8, 128, 1024).transpose(1, 0, 2).reshape(128, 8192)
        )
        wv = (
            W_qkv[:, v_cols].reshape(8, 128, 512).transpose(1, 0, 2).reshape(128, 4096)
        )
        wproj = (
            W_proj[hg * 512 : (hg + 1) * 512, :]
            .reshape(4, 128, 1024).transpose(1, 0, 2).reshape(128, 4096)
        )
        in_maps.append(
            {
                "xt": np.ascontiguousarray(xt.astype(ml_dtypes.bfloat16)),
                "wqk": np.ascontiguousarray(wqk.astype(ml_dtypes.bfloat16)),
                "bqk": np.ascontiguousarray(b_qkv[qk_cols], dtype=np.float32),
                "wv": np.ascontiguousarray(wv.astype(ml_dtypes.bfloat16)),
                "bv": np.ascontiguousarray(
                    b_qkv[v_cols].reshape(1, 512), dtype=np.float32
                ),
                "wproj": np.ascontiguousarray(wproj.astype(ml_dtypes.bfloat16)),
            }
        )
    return in_maps


_NC = None


def kernel(x, W_qkv, b_qkv, W_proj, b_proj, _trace=False):
    global _NC
    x = np.asarray(x, dtype=np.float32)
    W_qkv = np.asarray(W_qkv, dtype=np.float32)
    b_qkv = np.asarray(b_qkv, dtype=np.float32)
    W_proj = np.asarray(W_proj, dtype=np.float32)
    b_proj = np.asarray(b_proj, dtype=np.float32)

    in_maps = _shard_inputs(x, W_qkv, b_qkv, W_proj)
    if _NC is None:
        _NC = build_kernel()
    res = run_bass_kernel_spmd(
        _NC, in_maps, core_ids=list(range(8)), trace=_trace,
        trace_cores=list(range(8)) if _trace else None,
    )
    out = np.empty((B, T, C), dtype=np.float32)
    for b in range(B):
        out[b] = (
            res.results[2 * b]["y"].astype(np.float32)
            + res.results[2 * b + 1]["y"].astype(np.float32)
            + b_proj
        )
    if _trace:
        return out, res
    return out


# revision 43
# speedup vs baseline: 1.1932x; 1.1932x over previous
"""Multi-head causal self-attention (B=4, T=2048, C=1024, H=16) on 8 TRN2 cores.

Sharding: core c handles batch b = c//2 and head-group hg = c%2 (8 heads as 4
pairs): data parallel over B, tensor parallel over H. Pipelined schedule: qkv
projection runs in four 512-column t-chunks; attention round qc starts as soon
as chunk qc is done, so the softmax exp (ScalarE) overlaps the remaining
projection matmuls. Scores use row-tiled K=64 matmul pairs (head A on
partitions 0:64, head B on 64:128) sharing one 512-column q stream — both run
concurrently in the PE array. Exp is batched: one ACTIVATE covers a 4-bank
[128, 2048] PSUM tile (2 key-blocks x 2 heads). Softmax denominator comes from
a ones-row fused into the attn@V lhsT; normalization uses
reciprocal_approx_fast + per-pair K=2 broadcast matmuls.
"""

from collections import deque
from contextlib import ExitStack

import ml_dtypes
import numpy as np

import concourse.bass as bass
import concourse.bacc as bacc
import concourse.mybir as mybir
import concourse.tile as tile
from concourse.bass_utils import run_bass_kernel_spmd
from concourse.masks import make_upper_triangular

B, T, C, H, HS = 4, 2048, 1024, 16, 64
P = 128
NKB = T // P            # key blocks of 128
SCALE = HS ** -0.5

F32 = mybir.dt.float32
F32R = mybir.dt.float32r
BF16 = mybir.dt.bfloat16
Exp = mybir.ActivationFunctionType.Exp


def build_kernel():
    nc = bacc.Bacc("TRN2", target_bir_lowering=False)

    # all inputs pre-swizzled on host into SBUF layout: straight row-contiguous
    # copies with large DMA descriptors (fast per-queue transfer)
    xt_d = nc.dram_tensor("xt", (P, 4 * 4096), BF16, kind="ExternalInput")
    wqk_d = nc.dram_tensor("wqk", (P, 8 * 1024), BF16, kind="ExternalInput")
    bqk_d = nc.dram_tensor("bqk", (8 * P,), F32, kind="ExternalInput")
    wv_d = nc.dram_tensor("wv", (P, 8 * 512), BF16, kind="ExternalInput")
    bv_d = nc.dram_tensor("bv", (1, 512), F32R, kind="ExternalInput")
    wproj_d = nc.dram_tensor("wproj", (P, 4 * C), BF16, kind="ExternalInput")
    y_d = nc.dram_tensor("y", (T, C), BF16, kind="ExternalOutput")

    with tile.TileContext(nc) as tc, ExitStack() as big:
        const = big.enter_context(tc.tile_pool(name="const", bufs=1))
        persist = big.enter_context(tc.tile_pool(name="persist", bufs=1))
        xtp = big.enter_context(tc.tile_pool(name="xtp", bufs=2))
        atp = big.enter_context(tc.tile_pool(name="atp", bufs=3))
        rzp = big.enter_context(tc.tile_pool(name="rzp", bufs=2))
        rbp = big.enter_context(tc.tile_pool(name="rbp", bufs=2))
        ysp = big.enter_context(tc.tile_pool(name="ysp", bufs=3))
        ps_s = big.enter_context(tc.tile_pool(name="ps_s", bufs=2, space="PSUM"))
        ps_po = big.enter_context(tc.tile_pool(name="ps_po", bufs=1, space="PSUM"))
        ps_acc = big.enter_context(tc.tile_pool(name="ps_acc", bufs=2, space="PSUM"))

        # ---------------- input DMAs first: no waits, big transfers --------
        wqk_sb = persist.tile([P, 8 * 1024], BF16, tag="wqk")
        wv_sb = persist.tile([P, 8 * 512], BF16, tag="wv")
        wpj = persist.tile([P, 4 * C], BF16, tag="wpj")
        bqk = persist.tile([P, 8], F32, tag="bqk")
        bias_v = persist.tile([P, 512], F32, tag="bias_v")
        bvr = const.tile([1, 512], F32R, tag="bvr")
        xtc = [None] * 4

        def load_chunk(tck, ways=2):
            xtc[tck] = xtp.tile([P, 8 * 512], BF16, tag="xT", name=f"xT{tck}")
            rows = P // ways
            for w in range(ways):
                nc.sync.dma_start(
                    xtc[tck][w * rows : (w + 1) * rows, :],
                    xt_d[w * rows : (w + 1) * rows, tck * 4096 : (tck + 1) * 4096],
                )

        # x chunk0 and wv first (v groups can start before wqk finishes)
        load_chunk(0, ways=4)
        for w in range(2):
            nc.sync.dma_start(
                wv_sb[w * 64 : (w + 1) * 64, :], wv_d[w * 64 : (w + 1) * 64, :]
            )
        nc.sync.dma_start(bvr[:], bv_d[:])
        for w in range(4):
            nc.sync.dma_start(
                wqk_sb[w * 32 : (w + 1) * 32, :], wqk_d[w * 32 : (w + 1) * 32, :]
            )
        nc.sync.dma_start(bqk[:], bqk_d[:].rearrange("(a p) -> p a", p=P))

        # ---------------- constants ----------------
        mask = const.tile([P, P], BF16, tag="mask")
        make_upper_triangular(nc, mask[:], val=1.0, diag=True)
        ones_f = const.tile([P, P], F32, tag="ones_f")
        nc.vector.memset(ones_f[:], 1.0)
        ones_t = const.tile([1, P], F32R, tag="ones")
        nc.vector.tensor_copy(ones_t[:], ones_f[0:1, :])
        # selA rows {32p}: cols 0:64 = 1; selB rows {32p}: cols 64:128 = 1
        ones_b = const.tile([P, P], BF16, tag="ones_b")
        nc.vector.memset(ones_b[:], 1.0)
        selA = const.tile([P, P], BF16, tag="selA")
        selB = const.tile([P, P], BF16, tag="selB")
        nc.vector.memset(selA[:], 0.0)
        nc.vector.memset(selB[:], 0.0)
        for pr in range(4):
            nc.sync.dma_start(
                selA[pr * 32 : pr * 32 + 1, 0:64], ones_b[0:1, 0:64]
            )
            nc.sync.dma_start(
                selB[pr * 32 : pr * 32 + 1, 64:P], ones_b[0:1, 0:64]
            )

        # ---------------- persistent tensors ----------------
        # qk_all: per pair p: block 2p = q (rows 0:64 head A, 64:128 head B),
        # block 2p+1 = k (same row split). [P, 8*T] bf16.
        qk_all = persist.tile([P, 8 * T], BF16, tag="qk")
        # v_all: per (pair, kb): [vA(64) | onesA(1) | vB(64) | onesB(1)] = 130
        v_all = persist.tile([P, 4 * NKB * 130], BF16, tag="v")
        va4 = v_all[:].rearrange("p (a b c) -> p a b c", a=4, b=NKB, c=130)
        nc.vector.tensor_copy(va4[:, :, :, 64:65], ones_f[:, 0 : 4 * NKB])
        nc.vector.tensor_copy(va4[:, :, :, 129:130], ones_f[:, 0 : 4 * NKB])
        # unnormalized attention output, pair-stacked transposed layout
        aoT = persist.tile([P, 4 * T], BF16, tag="aoT")
        # bias_v[128, 512] = b_v broadcast along partitions (K=1 matmul)
        pbv = ps_acc.tile([P, 512], F32, tag="acc")
        nc.tensor.matmul(pbv[:], ones_t[:], bvr[:], start=True, stop=True)
        nc.vector.tensor_copy(bias_v[:], pbv[:])

        # ---------------- work-item generators ----------------
        def qk_group(tck, chb):
            def run():
                pq = ps_acc.tile([P, 512], F32, tag="acc")
                for cb in range(8):
                    nc.tensor.matmul(
                        pq[:],
                        wqk_sb[:, cb * 1024 + chb * P : cb * 1024 + (chb + 1) * P],
                        xtc[tck][:, cb * 512 : (cb + 1) * 512],
                        start=(cb == 0),
                        stop=(cb == 7),
                    )
                p_pair, kind = chb // 2, chb % 2  # kind: 0 = q, 1 = k
                blk = 2 * p_pair + kind
                nc.vector.tensor_scalar_add(
                    qk_all[:, blk * T + tck * 512 : blk * T + (tck + 1) * 512],
                    pq[:],
                    bqk[:, chb : chb + 1],
                )
            return run

        def v_group(tck, tb):
            def run():
                kb = tck * 4 + tb
                pv = ps_acc.tile([P, 512], F32, tag="acc")
                for cb in range(8):
                    nc.tensor.matmul(
                        pv[:],
                        xtc[tck][:, cb * 512 + tb * P : cb * 512 + (tb + 1) * P],
                        wv_sb[:, cb * 512 : (cb + 1) * 512],
                        start=(cb == 0),
                        stop=(cb == 7),
                    )
                dst = bass.AP(
                    v_all[:].tensor,
                    v_all[:].offset + kb * 130,
                    [[v_all[:].ap[0][0], P], [NKB * 130, 4], [65, 2], [1, 64]],
                )
                src = bass.AP(
                    pv[:].tensor,
                    pv[:].offset,
                    [[pv[:].ap[0][0], P], [128, 4], [64, 2], [1, 64]],
                )
                bsrc = bass.AP(
                    bias_v[:].tensor,
                    bias_v[:].offset,
                    [[bias_v[:].ap[0][0], P], [128, 4], [64, 2], [1, 64]],
                )
                nc.vector.tensor_tensor(dst, src, bsrc, mybir.AluOpType.add)
            return run

        def chunk_items(tck):
            items = [qk_group(tck, chb) for chb in range(8)]
            items += [v_group(tck, tb) for tb in range(4)]
            return items

        def proj_group(qc, tb, oc):
            def run():
                py = ps_acc.tile([P, 512], F32, tag="acc")
                for pp in range(4):
                    nc.tensor.matmul(
                        py[:],
                        aoT[:, pp * T + tb * P : pp * T + (tb + 1) * P],
                        wpj[:, pp * C + oc * 512 : pp * C + (oc + 1) * 512],
                        start=(pp == 0),
                        stop=(pp == 3),
                    )
                ys = ysp.tile([P, 512], BF16, tag="ys")
                nc.vector.tensor_copy(ys[:], py[:])
                nc.sync.dma_start(
                    y_d[tb * P : (tb + 1) * P, oc * 512 : (oc + 1) * 512], ys[:]
                )
            return run

        def proj_items(qc):
            return [
                proj_group(qc, tb, oc)
                for tb in range(qc * 4, (qc + 1) * 4)
                for oc in range(2)
            ]

        def norm_pair(qc, rz, p_pair):
            # rz row 32p holds [Z_A (512) | Z_B (512)] for pair p. Two
            # accumulating K=1 broadcast matmuls -> pbt (rows 0:64 = Z_A,
            # 64:128 = Z_B), reciprocal after broadcast, one [128,512] mul.
            r = p_pair * 32
            pbt = ps_acc.tile([P, 512], F32, tag="acc")
            nc.tensor.matmul(
                pbt[:],
                selA[r : r + 1, :],
                rz[r : r + 1, 0:512],
                start=True,
                stop=False,
                tile_position=(r, 0),
            )
            nc.tensor.matmul(
                pbt[:],
                selB[r : r + 1, :],
                rz[r : r + 1, 512:1024],
                start=False,
                stop=True,
                tile_position=(r, 0),
            )
            rb = rbp.tile([P, 512], F32, tag="rb")
            nc.vector.reciprocal_approx_fast(rb[:], pbt[:])
            col = p_pair * T + qc * 512
            nc.vector.tensor_mul(
                aoT[:, col : col + 512], aoT[:, col : col + 512], rb[:]
            )

        # ---------------- attention rounds with static interleave ----------
        fill_q = deque()

        def drain(n):
            for _ in range(n):
                if fill_q:
                    fill_q.popleft()()

        def run_round(qc, rz, prereq=None, late_prereq=None):
            nkb = 4 * qc + 4
            n_groups_total = 4 * nkb
            fill_total = len(fill_q)
            done_fill = [0]
            done_groups = [0]

            def after_group():
                done_groups[0] += 1
                want = fill_total * done_groups[0] // n_groups_total
                d = want - done_fill[0]
                done_fill[0] += d
                drain(d)

            pending_norm = [None]
            for p_pair in range(4):
                if prereq is not None:
                    for it in prereq(p_pair):
                        it()
                qblk, kblk = 2 * p_pair, 2 * p_pair + 1
                po0 = ps_po.tile([65, 512], F32, tag="po0")
                po1 = ps_po.tile([65, 512], F32, tag="po1")
                po = [po0, po1]
                pend = None  # (at, kb)
                for kb in range(nkb):
                    if late_prereq is not None:
                        for it in late_prereq(p_pair, kb):
                            it()
                    qoff = max(0, kb * P - qc * 512)
                    S = ps_s.tile([P, 1024], F32, tag="S")
                    at = atp.tile([P, 1024], BF16, tag="at")
                    for hh in range(2):
                        r0 = hh * 64
                        nc.tensor.matmul(
                            S[:, hh * 512 + qoff : (hh + 1) * 512],
                            qk_all[r0 : r0 + 64, kblk * T + kb * P : kblk * T + (kb + 1) * P],
                            qk_all[r0 : r0 + 64, qblk * T + qc * 512 + qoff : qblk * T + (qc + 1) * 512],
                            start=True,
                            stop=True,
                        )
                    nc.scalar.activation(at[:], S[:], Exp, scale=SCALE)
                    if kb * P >= qc * 512:  # diagonal: zero k > q entries
                        # on GpSimd: keeps the Vector queue free for evictions
                        for hh in range(2):
                            c0 = hh * 512 + qoff
                            nc.gpsimd.tensor_tensor(
                                at[:, c0 : c0 + P], at[:, c0 : c0 + P], mask[:],
                                mybir.AluOpType.mult,
                            )
                    if pend is not None:
                        emit_attnv(qc, p_pair, po, *pend, nkb)
                    pend = (at, kb)
                    after_group()
                    if pending_norm[0] is not None:
                        # previous pair's normalization, off the critical path
                        norm_pair(qc, rz, pending_norm[0])
                        pending_norm[0] = None
                emit_attnv(qc, p_pair, po, *pend, nkb)
                # evict raw ao + Z rows, then normalize this pair inline
                col = p_pair * T + qc * 512
                nc.vector.tensor_copy(aoT[0:64, col : col + 512], po0[0:64, :])
                nc.vector.tensor_copy(aoT[64:P, col : col + 512], po1[0:64, :])
                r = p_pair * 32
                nc.vector.tensor_copy(rz[r : r + 1, 0:512], po0[64:65, :])
                nc.vector.tensor_copy(rz[r : r + 1, 512:1024], po1[64:65, :])
                pending_norm[0] = p_pair
            drain(len(fill_q))
            norm_pair(qc, rz, pending_norm[0])

        def emit_attnv(qc, p_pair, po, at, kb, nkb):
            qoff = max(0, kb * P - qc * 512)
            for hh in range(2):
                nc.tensor.matmul(
                    po[hh][:, qoff:512],
                    v_all[:, p_pair * NKB * 130 + kb * 130 + hh * 65 :
                          p_pair * NKB * 130 + kb * 130 + hh * 65 + 65],
                    at[:, hh * 512 + qoff : (hh + 1) * 512],
                    start=(kb == 0),
                    stop=(kb == nkb - 1),
                    skip_group_check=True,
                )

        # ---------------- main schedule ----------------
        # round 0 pair p only needs its own q/k groups (chb 2p, 2p+1) and the
        # first four v groups; emit the minimum up front, rest per-pair.
        for it in [v_group(0, tb) for tb in range(4)] + [qk_group(0, 0), qk_group(0, 1)]:
            it()

        def prereq0(p_pair):
            if p_pair == 0:
                return []
            return [qk_group(0, 2 * p_pair), qk_group(0, 2 * p_pair + 1)]

        # round 3's k and v chunk-3 groups are deferred INTO round 3 (they are
        # only needed from kb 12 onward) to fill its ACT-paced stalls.
        def late3(p_pair, kb):
            items = []
            if kb == 12:
                items.append(qk_group(3, 2 * p_pair + 1))
                if p_pair == 0:
                    items.append(v_group(3, 0))
            elif p_pair == 0 and kb in (13, 14, 15):
                items.append(v_group(3, kb - 12))
            return items

        rzs = [None] * 4
        for qc in range(4):
            rzs[qc] = rzp.tile([P, 1024], BF16, tag="rz", name=f"rz{qc}")
            if qc < 3:
                fill_q.append(lambda t=qc + 1: load_chunk(t))
            if qc == 0:
                def load_wpj():
                    for w in range(2):
                        nc.sync.dma_start(
                            wpj[w * 64 : (w + 1) * 64, :],
                            wproj_d[w * 64 : (w + 1) * 64, :],
                        )
                fill_q.append(load_wpj)
            if qc < 2:
                fill_q.extend(chunk_items(qc + 1))
            elif qc == 2:
                # only chunk 3's q groups (k/v deferred into round 3)
                fill_q.extend(qk_group(3, 2 * p) for p in range(4))
                fill_q.extend(proj_items(0))
                fill_q.extend(proj_items(1))
            if qc == 3:
                fill_q.extend(proj_items(2))
            run_round(
                qc,
                rzs[qc],
                prereq=prereq0 if qc == 0 else None,
                late_prereq=late3 if qc == 3 else None,
            )
        for it in proj_items(3):
            it()

    nc.compile()
    return nc


def _shard_inputs(x, W_qkv, b_qkv, W_proj):
    """Build the 8 per-core input maps."""
    in_maps = []
    for c in range(8):
        b = c // 2
        hg = c % 2
        heads = [hg * 8 + j for j in range(8)]
        qk_cols = []
        for p in range(4):
            ha, hb = heads[2 * p], heads[2 * p + 1]
            for part in range(2):  # q, k
                qk_cols.extend(range(ha * 192 + part * 64, ha * 192 + part * 64 + 64))
                qk_cols.extend(range(hb * 192 + part * 64, hb * 192 + part * 64 + 64))
        qk_cols = np.array(qk_cols)
        v_cols = []
        for p in range(4):
            ha, hb = heads[2 * p], heads[2 * p + 1]
            v_cols.extend(range(ha * 192 + 128, ha * 192 + 192))
            v_cols.extend(range(hb * 192 + 128, hb * 192 + 192))
        v_cols = np.array(v_cols)
        # pre-swizzle into the exact SBUF layouts (row-contiguous for fast DMA)
        # xt[p, tck*4096 + cb*512 + t'] = x[b][tck*512 + t', cb*128 + p]
        xt = (
            x[b].reshape(4, 512, 8, 128).transpose(3, 0, 2, 1).reshape(128, 16384)
        )
        # wqk[p, cb*1024 + ch] = W_qkv[cb*128 + p, qk_cols[ch]]
        wqk = (
            W_qkv[:, qk_cols].reshape(8, 128, 1024).transpose(1, 0, 2).reshape(128, 8192)
        )
        wv = (
            W_qkv[:, v_cols].reshape(8, 128, 512).transpose(1, 0, 2).reshape(128, 4096)
        )
        wproj = (
            W_proj[hg * 512 : (hg + 1) * 512, :]
            .reshape(4, 128, 1024).transpose(1, 0, 2).reshape(128, 4096)
        )
        in_maps.append(
            {
                "xt": np.ascontiguousarray(xt.astype(ml_dtypes.bfloat16)),
                "wqk": np.ascontiguousarray(wqk.astype(ml_dtypes.bfloat16)),
                "bqk": np.ascontiguousarray(b_qkv[qk_cols], dtype=np.float32),
                "wv": np.ascontiguousarray(wv.astype(ml_dtypes.bfloat16)),
                "bv": np.ascontiguousarray(
                    b_qkv[v_cols].reshape(1, 512), dtype=np.float32
                ),
                "wproj": np.ascontiguousarray(wproj.astype(ml_dtypes.bfloat16)),
            }
        )
    return in_maps


_NC = None


def kernel(x, W_qkv, b_qkv, W_proj, b_proj, _trace=False):
    global _NC
    x = np.asarray(x, dtype=np.float32)
    W_qkv = np.asarray(W_qkv, dtype=np.float32)
    b_qkv = np.asarray(b_qkv, dtype=np.float32)
    W_proj = np.asarray(W_proj, dtype=np.float32)
    b_proj = np.asarray(b_proj, dtype=np.float32)

    in_maps = _shard_inputs(x, W_qkv, b_qkv, W_proj)
    if _NC is None:
        _NC = build_kernel()
    res = run_bass_kernel_spmd(
        _NC, in_maps, core_ids=list(range(8)), trace=_trace,
        trace_cores=list(range(8)) if _trace else None,
    )
    out = np.empty((B, T, C), dtype=np.float32)
    for b in range(B):
        out[b] = (
            res.results[2 * b]["y"].astype(np.float32)
            + res.results[2 * b + 1]["y"].astype(np.float32)
            + b_proj
        )
    if _trace:
        return out, res
    return out


# revision 47
# speedup vs baseline: 1.2356x; 1.0356x over previous
"""Multi-head causal self-attention (B=4, T=2048, C=1024, H=16) on 8 TRN2 cores.

Sharding: core c handles batch b = c//2 and head-group hg = c%2 (8 heads as 4
pairs): data parallel over B, tensor parallel over H. Pipelined schedule: qkv
projection runs in four 512-column t-chunks; attention round qc starts as soon
as chunk qc is done, so the softmax exp (ScalarE) overlaps the remaining
projection matmuls. Scores use row-tiled K=64 matmul pairs (head A on
partitions 0:64, head B on 64:128) sharing one 512-column q stream — both run
concurrently in the PE array. Exp is batched: one ACTIVATE covers a 4-bank
[128, 2048] PSUM tile (2 key-blocks x 2 heads). Softmax denominator comes from
a ones-row fused into the attn@V lhsT; normalization uses
reciprocal_approx_fast + per-pair K=2 broadcast matmuls.
"""

from collections import deque
from contextlib import ExitStack

import ml_dtypes
import numpy as np

import concourse.bass as bass
import concourse.bacc as bacc
import concourse.mybir as mybir
import concourse.tile as tile
from concourse.bass_utils import run_bass_kernel_spmd
from concourse.masks import make_upper_triangular

B, T, C, H, HS = 4, 2048, 1024, 16, 64
P = 128
NKB = T // P            # key blocks of 128
SCALE = HS ** -0.5

F32 = mybir.dt.float32
F32R = mybir.dt.float32r
BF16 = mybir.dt.bfloat16
Exp = mybir.ActivationFunctionType.Exp


def build_kernel():
    nc = bacc.Bacc("TRN2", target_bir_lowering=False)

    # all inputs pre-swizzled on host into SBUF layout: straight row-contiguous
    # copies with large DMA descriptors (fast per-queue transfer)
    xt_d = nc.dram_tensor("xt", (P, 4 * 4096), BF16, kind="ExternalInput")
    wqk_d = nc.dram_tensor("wqk", (P, 8 * 1024), BF16, kind="ExternalInput")
    bqk_d = nc.dram_tensor("bqk", (8 * P,), F32, kind="ExternalInput")
    wv_d = nc.dram_tensor("wv", (P, 8 * 512), BF16, kind="ExternalInput")
    bv_d = nc.dram_tensor("bv", (1, 512), F32R, kind="ExternalInput")
    wproj_d = nc.dram_tensor("wproj", (P, 4 * C), BF16, kind="ExternalInput")
    y_d = nc.dram_tensor("y", (T, C), BF16, kind="ExternalOutput")

    with tile.TileContext(nc) as tc, ExitStack() as big:
        const = big.enter_context(tc.tile_pool(name="const", bufs=1))
        persist = big.enter_context(tc.tile_pool(name="persist", bufs=1))
        xtp = big.enter_context(tc.tile_pool(name="xtp", bufs=2))
        atp = big.enter_context(tc.tile_pool(name="atp", bufs=3))
        rzp = big.enter_context(tc.tile_pool(name="rzp", bufs=2))
        rbp = big.enter_context(tc.tile_pool(name="rbp", bufs=2))
        ysp = big.enter_context(tc.tile_pool(name="ysp", bufs=3))
        ps_s = big.enter_context(tc.tile_pool(name="ps_s", bufs=2, space="PSUM"))
        ps_po = big.enter_context(tc.tile_pool(name="ps_po", bufs=1, space="PSUM"))
        ps_acc = big.enter_context(tc.tile_pool(name="ps_acc", bufs=2, space="PSUM"))

        # ---------------- input DMAs first: no waits, big transfers --------
        wqk_sb = persist.tile([P, 8 * 1024], BF16, tag="wqk")
        wv_sb = persist.tile([P, 8 * 512], BF16, tag="wv")
        wpj = persist.tile([P, 4 * C], BF16, tag="wpj")
        bqk = persist.tile([P, 8], F32, tag="bqk")
        bias_v = persist.tile([P, 512], F32, tag="bias_v")
        bvr = const.tile([1, 512], F32R, tag="bvr")
        xtc = [None] * 4

        def load_chunk(tck, ways=2):
            xtc[tck] = xtp.tile([P, 8 * 512], BF16, tag="xT", name=f"xT{tck}")
            rows = P // ways
            for w in range(ways):
                nc.sync.dma_start(
                    xtc[tck][w * rows : (w + 1) * rows, :],
                    xt_d[w * rows : (w + 1) * rows, tck * 4096 : (tck + 1) * 4096],
                )

        # x chunk0 and wv first (v groups can start before wqk finishes)
        load_chunk(0, ways=4)
        for w in range(2):
            nc.sync.dma_start(
                wv_sb[w * 64 : (w + 1) * 64, :], wv_d[w * 64 : (w + 1) * 64, :]
            )
        nc.sync.dma_start(bvr[:], bv_d[:])
        for w in range(4):
            nc.sync.dma_start(
                wqk_sb[w * 32 : (w + 1) * 32, :], wqk_d[w * 32 : (w + 1) * 32, :]
            )
        nc.sync.dma_start(bqk[:], bqk_d[:].rearrange("(a p) -> p a", p=P))

        # ---------------- constants ----------------
        mask = const.tile([P, P], BF16, tag="mask")
        make_upper_triangular(nc, mask[:], val=1.0, diag=True)
        ones_f = const.tile([P, P], F32, tag="ones_f")
        nc.vector.memset(ones_f[:], 1.0)
        ones_t = const.tile([1, P], F32R, tag="ones")
        nc.vector.tensor_copy(ones_t[:], ones_f[0:1, :])
        # selA rows {32p}: cols 0:64 = 1; selB rows {32p}: cols 64:128 = 1
        ones_b = const.tile([P, P], BF16, tag="ones_b")
        nc.vector.memset(ones_b[:], 1.0)
        selA = const.tile([P, P], BF16, tag="selA")
        selB = const.tile([P, P], BF16, tag="selB")
        nc.vector.memset(selA[:], 0.0)
        nc.vector.memset(selB[:], 0.0)
        for pr in range(4):
            nc.sync.dma_start(
                selA[pr * 32 : pr * 32 + 1, 0:64], ones_b[0:1, 0:64]
            )
            nc.sync.dma_start(
                selB[pr * 32 : pr * 32 + 1, 64:P], ones_b[0:1, 0:64]
            )

        # ---------------- persistent tensors ----------------
        # qk_all: per pair p: block 2p = q (rows 0:64 head A, 64:128 head B),
        # block 2p+1 = k (same row split). [P, 8*T] bf16.
        qk_all = persist.tile([P, 8 * T], BF16, tag="qk")
        # v_all: per (pair, kb): [vA(64) | onesA(1) | vB(64) | onesB(1)] = 130
        v_all = persist.tile([P, 4 * NKB * 130], BF16, tag="v")
        va4 = v_all[:].rearrange("p (a b c) -> p a b c", a=4, b=NKB, c=130)
        nc.vector.tensor_copy(va4[:, :, :, 64:65], ones_f[:, 0 : 4 * NKB])
        nc.vector.tensor_copy(va4[:, :, :, 129:130], ones_f[:, 0 : 4 * NKB])
        # unnormalized attention output, pair-stacked transposed layout
        aoT = persist.tile([P, 4 * T], BF16, tag="aoT")
        # bias_v[128, 512] = b_v broadcast along partitions (K=1 matmul)
        pbv = ps_acc.tile([P, 512], F32, tag="acc")
        nc.tensor.matmul(pbv[:], ones_t[:], bvr[:], start=True, stop=True)
        nc.vector.tensor_copy(bias_v[:], pbv[:])

        # ---------------- work-item generators ----------------
        def qk_group(tck, chb):
            def run():
                pq = ps_acc.tile([P, 512], F32, tag="acc")
                for cb in range(8):
                    nc.tensor.matmul(
                        pq[:],
                        wqk_sb[:, cb * 1024 + chb * P : cb * 1024 + (chb + 1) * P],
                        xtc[tck][:, cb * 512 : (cb + 1) * 512],
                        start=(cb == 0),
                        stop=(cb == 7),
                    )
                p_pair, kind = chb // 2, chb % 2  # kind: 0 = q, 1 = k
                blk = 2 * p_pair + kind
                nc.vector.tensor_scalar_add(
                    qk_all[:, blk * T + tck * 512 : blk * T + (tck + 1) * 512],
                    pq[:],
                    bqk[:, chb : chb + 1],
                )
            return run

        def v_group(tck, tb):
            def run():
                kb = tck * 4 + tb
                pv = ps_acc.tile([P, 512], F32, tag="acc")
                for cb in range(8):
                    nc.tensor.matmul(
                        pv[:],
                        xtc[tck][:, cb * 512 + tb * P : cb * 512 + (tb + 1) * P],
                        wv_sb[:, cb * 512 : (cb + 1) * 512],
                        start=(cb == 0),
                        stop=(cb == 7),
                    )
                dst = bass.AP(
                    v_all[:].tensor,
                    v_all[:].offset + kb * 130,
                    [[v_all[:].ap[0][0], P], [NKB * 130, 4], [65, 2], [1, 64]],
                )
                src = bass.AP(
                    pv[:].tensor,
                    pv[:].offset,
                    [[pv[:].ap[0][0], P], [128, 4], [64, 2], [1, 64]],
                )
                bsrc = bass.AP(
                    bias_v[:].tensor,
                    bias_v[:].offset,
                    [[bias_v[:].ap[0][0], P], [128, 4], [64, 2], [1, 64]],
                )
                nc.vector.tensor_tensor(dst, src, bsrc, mybir.AluOpType.add)
            return run

        def chunk_items(tck):
            items = [qk_group(tck, chb) for chb in range(8)]
            items += [v_group(tck, tb) for tb in range(4)]
            return items

        def proj_group(qc, tb, oc):
            def run():
                py = ps_acc.tile([P, 512], F32, tag="acc")
                for pp in range(4):
                    nc.tensor.matmul(
                        py[:],
                        aoT[:, pp * T + tb * P : pp * T + (tb + 1) * P],
                        wpj[:, pp * C + oc * 512 : pp * C + (oc + 1) * 512],
                        start=(pp == 0),
                        stop=(pp == 3),
                    )
                ys = ysp.tile([P, 512], BF16, tag="ys")
                nc.vector.tensor_copy(ys[:], py[:])
                nc.sync.dma_start(
                    y_d[tb * P : (tb + 1) * P, oc * 512 : (oc + 1) * 512], ys[:]
                )
            return run

        def proj_items(qc):
            return [
                proj_group(qc, tb, oc)
                for tb in range(qc * 4, (qc + 1) * 4)
                for oc in range(2)
            ]

        def norm_pair(qc, rz, p_pair):
            # rz row 32p holds [Z_A (512) | Z_B (512)] for pair p. Two
            # accumulating K=1 broadcast matmuls -> pbt (rows 0:64 = Z_A,
            # 64:128 = Z_B), reciprocal after broadcast, one [128,512] mul.
            r = p_pair * 32
            pbt = ps_acc.tile([P, 512], F32, tag="acc")
            nc.tensor.matmul(
                pbt[:],
                selA[r : r + 1, :],
                rz[r : r + 1, 0:512],
                start=True,
                stop=False,
                tile_position=(r, 0),
            )
            nc.tensor.matmul(
                pbt[:],
                selB[r : r + 1, :],
                rz[r : r + 1, 512:1024],
                start=False,
                stop=True,
                tile_position=(r, 0),
            )
            rb = rbp.tile([P, 512], F32, tag="rb")
            nc.vector.reciprocal_approx_fast(rb[:], pbt[:])
            col = p_pair * T + qc * 512
            nc.vector.tensor_mul(
                aoT[:, col : col + 512], aoT[:, col : col + 512], rb[:]
            )

        # ---------------- attention rounds with static interleave ----------
        fill_q = deque()

        def drain(n):
            for _ in range(n):
                if fill_q:
                    fill_q.popleft()()

        def run_round(qc, rz, prereq=None, late_prereq=None):
            nkb = 4 * qc + 4
            n_groups_total = 4 * nkb
            fill_total = len(fill_q)
            done_fill = [0]
            done_groups = [0]

            def after_group():
                done_groups[0] += 1
                want = fill_total * done_groups[0] // n_groups_total
                d = want - done_fill[0]
                done_fill[0] += d
                drain(d)

            pending_norm = [None]
            for p_pair in range(4):
                if prereq is not None:
                    for it in prereq(p_pair):
                        it()
                qblk, kblk = 2 * p_pair, 2 * p_pair + 1
                po0 = ps_po.tile([65, 512], F32, tag="po0")
                po1 = ps_po.tile([65, 512], F32, tag="po1")
                po = [po0, po1]
                pend = None  # (at, kb)
                # diagonal blocks first: their masks/GpSimd work lands while
                # the vector queue is free, and the pair tail stays clean
                kb_order = list(range(4 * qc, nkb)) + list(range(0, 4 * qc))
                first_kb, last_kb = kb_order[0], kb_order[-1]
                for kb in kb_order:
                    if late_prereq is not None:
                        for it in late_prereq(p_pair, kb):
                            it()
                    qoff = max(0, kb * P - qc * 512)
                    S = ps_s.tile([P, 1024], F32, tag="S")
                    at = atp.tile([P, 1024], BF16, tag="at")
                    for hh in range(2):
                        r0 = hh * 64
                        nc.tensor.matmul(
                            S[:, hh * 512 + qoff : (hh + 1) * 512],
                            qk_all[r0 : r0 + 64, kblk * T + kb * P : kblk * T + (kb + 1) * P],
                            qk_all[r0 : r0 + 64, qblk * T + qc * 512 + qoff : qblk * T + (qc + 1) * 512],
                            start=True,
                            stop=True,
                        )
                    nc.scalar.activation(at[:], S[:], Exp, scale=SCALE)
                    if kb * P >= qc * 512:  # diagonal: zero k > q entries
                        # on GpSimd: keeps the Vector queue free for evictions
                        for hh in range(2):
                            c0 = hh * 512 + qoff
                            nc.gpsimd.tensor_tensor(
                                at[:, c0 : c0 + P], at[:, c0 : c0 + P], mask[:],
                                mybir.AluOpType.mult,
                            )
                    if pend is not None:
                        emit_attnv(qc, p_pair, po, *pend, first_kb, last_kb)
                    pend = (at, kb)
                    after_group()
                    if pending_norm[0] is not None:
                        # previous pair's normalization, off the critical path
                        norm_pair(qc, rz, pending_norm[0])
                        pending_norm[0] = None
                emit_attnv(qc, p_pair, po, *pend, first_kb, last_kb)
                # evict raw ao (ScalarE: its pair-boundary bubble) + Z rows
                col = p_pair * T + qc * 512
                nc.scalar.copy(aoT[0:64, col : col + 512], po0[0:64, :])
                nc.scalar.copy(aoT[64:P, col : col + 512], po1[0:64, :])
                r = p_pair * 32
                nc.vector.tensor_copy(rz[r : r + 1, 0:512], po0[64:65, :])
                nc.vector.tensor_copy(rz[r : r + 1, 512:1024], po1[64:65, :])
                pending_norm[0] = p_pair
            drain(len(fill_q))
            norm_pair(qc, rz, pending_norm[0])

        def emit_attnv(qc, p_pair, po, at, kb, first_kb, last_kb):
            qoff = max(0, kb * P - qc * 512)
            for hh in range(2):
                nc.tensor.matmul(
                    po[hh][:, qoff:512],
                    v_all[:, p_pair * NKB * 130 + kb * 130 + hh * 65 :
                          p_pair * NKB * 130 + kb * 130 + hh * 65 + 65],
                    at[:, hh * 512 + qoff : (hh + 1) * 512],
                    start=(kb == first_kb),
                    stop=(kb == last_kb),
                    skip_group_check=True,
                )

        # ---------------- main schedule ----------------
        # round 0 pair p only needs its own q/k groups (chb 2p, 2p+1) and the
        # first four v groups; emit the minimum up front, rest per-pair.
        for it in [v_group(0, tb) for tb in range(4)] + [qk_group(0, 0), qk_group(0, 1)]:
            it()

        def prereq0(p_pair):
            if p_pair == 0:
                return []
            return [qk_group(0, 2 * p_pair), qk_group(0, 2 * p_pair + 1)]

        # round 3's k and v chunk-3 groups are deferred INTO round 3 (they are
        # only needed from kb 12 onward) to fill its ACT-paced stalls.
        def late3(p_pair, kb):
            items = []
            if kb == 12:
                items.append(qk_group(3, 2 * p_pair + 1))
                if p_pair == 0:
                    items.append(v_group(3, 0))
            elif p_pair == 0 and kb in (13, 14, 15):
                items.append(v_group(3, kb - 12))
            return items

        rzs = [None] * 4
        for qc in range(4):
            rzs[qc] = rzp.tile([P, 1024], BF16, tag="rz", name=f"rz{qc}")
            if qc < 3:
                fill_q.append(lambda t=qc + 1: load_chunk(t))
            if qc == 0:
                def load_wpj():
                    for w in range(2):
                        nc.sync.dma_start(
                            wpj[w * 64 : (w + 1) * 64, :],
                            wproj_d[w * 64 : (w + 1) * 64, :],
                        )
                fill_q.append(load_wpj)
            if qc < 2:
                fill_q.extend(chunk_items(qc + 1))
            elif qc == 2:
                # only chunk 3's q groups (k/v deferred into round 3)
                fill_q.extend(qk_group(3, 2 * p) for p in range(4))
                fill_q.extend(proj_items(0))
                fill_q.extend(proj_items(1))
            if qc == 3:
                fill_q.extend(proj_items(2))
            run_round(
                qc,
                rzs[qc],
                prereq=prereq0 if qc == 0 else None,
                late_prereq=late3 if qc == 3 else None,
            )
        for it in proj_items(3):
            it()

    nc.compile()
    return nc


def _shard_inputs(x, W_qkv, b_qkv, W_proj):
    """Build the 8 per-core input maps."""
    in_maps = []
    for c in range(8):
        b = c // 2
        hg = c % 2
        heads = [hg * 8 + j for j in range(8)]
        qk_cols = []
        for p in range(4):
            ha, hb = heads[2 * p], heads[2 * p + 1]
            for part in range(2):  # q, k
                qk_cols.extend(range(ha * 192 + part * 64, ha * 192 + part * 64 + 64))
                qk_cols.extend(range(hb * 192 + part * 64, hb * 192 + part * 64 + 64))
        qk_cols = np.array(qk_cols)
        v_cols = []
        for p in range(4):
            ha, hb = heads[2 * p], heads[2 * p + 1]
            v_cols.extend(range(ha * 192 + 128, ha * 192 + 192))
            v_cols.extend(range(hb * 192 + 128, hb * 192 + 192))
        v_cols = np.array(v_cols)
        # pre-swizzle into the exact SBUF layouts (row-contiguous for fast DMA)
        # xt[p, tck*4096 + cb*512 + t'] = x[b][tck*512 + t', cb*128 + p]
        xt = (
            x[b].reshape(4, 512, 8, 128).transpose(3, 0, 2, 1).reshape(128, 16384)
        )
        # wqk[p, cb*1024 + ch] = W_qkv[cb*128 + p, qk_cols[ch]]
        wqk = (
            W_qkv[:, qk_cols].reshape(8, 128, 1024).transpose(1, 0, 2).reshape(128, 8192)
        )
        wv = (
            W_qkv[:, v_cols].reshape(8, 128, 512).transpose(1, 0, 2).reshape(128, 4096)
        )
        wproj = (
            W_proj[hg * 512 : (hg + 1) * 512, :]
            .reshape(4, 128, 1024).transpose(1, 0, 2).reshape(128, 4096)
        )
        in_maps.append(
            {
                "xt": np.ascontiguousarray(xt.astype(ml_dtypes.bfloat16)),
                "wqk": np.ascontiguousarray(wqk.astype(ml_dtypes.bfloat16)),
                "bqk": np.ascontiguousarray(b_qkv[qk_cols], dtype=np.float32),
                "wv": np.ascontiguousarray(wv.astype(ml_dtypes.bfloat16)),
                "bv": np.ascontiguousarray(
                    b_qkv[v_cols].reshape(1, 512), dtype=np.float32
                ),
                "wproj": np.ascontiguousarray(wproj.astype(ml_dtypes.bfloat16)),
            }
        )
    return in_maps


_NC = None


def kernel(x, W_qkv, b_qkv, W_proj, b_proj, _trace=False):
    global _NC
    x = np.asarray(x, dtype=np.float32)
    W_qkv = np.asarray(W_qkv, dtype=np.float32)
    b_qkv = np.asarray(b_qkv, dtype=np.float32)
    W_proj = np.asarray(W_proj, dtype=np.float32)
    b_proj = np.asarray(b_proj, dtype=np.float32)

    in_maps = _shard_inputs(x, W_qkv, b_qkv, W_proj)
    if _NC is None:
        _NC = build_kernel()
    res = run_bass_kernel_spmd(
        _NC, in_maps, core_ids=list(range(8)), trace=_trace,
        trace_cores=list(range(8)) if _trace else None,
    )
    out = np.empty((B, T, C), dtype=np.float32)
    for b in range(B):
        out[b] = (
            res.results[2 * b]["y"].astype(np.float32)
            + res.results[2 * b + 1]["y"].astype(np.float32)
            + b_proj
        )
    if _trace:
        return out, res
    return out


# revision 66
# speedup vs baseline: 1.2842x; 1.0393x over previous
"""Multi-head causal self-attention (B=4, T=2048, C=1024, H=16) on 8 TRN2 cores.

Sharding: core c handles batch b = c//2 and head-group hg = c%2 (8 heads as 4
pairs): data parallel over B, tensor parallel over H. Pipelined schedule: qkv
projection runs in four 512-column t-chunks; attention round qc starts as soon
as chunk qc is done, so the softmax exp (ScalarE) overlaps the remaining
projection matmuls. Scores use row-tiled K=64 matmul pairs (head A on
partitions 0:64, head B on 64:128) sharing one 512-column q stream — both run
concurrently in the PE array. Exp is batched: one ACTIVATE covers a 4-bank
[128, 2048] PSUM tile (2 key-blocks x 2 heads). Softmax denominator comes from
a ones-row fused into the attn@V lhsT; normalization uses
reciprocal_approx_fast + per-pair K=2 broadcast matmuls.
"""

from collections import deque
from contextlib import ExitStack

import ml_dtypes
import numpy as np

import concourse.bass as bass
import concourse.bacc as bacc
import concourse.mybir as mybir
import concourse.tile as tile
from concourse.bass_utils import run_bass_kernel_spmd
from concourse.masks import make_upper_triangular

B, T, C, H, HS = 4, 2048, 1024, 16, 64
P = 128
NKB = T // P            # key blocks of 128
SCALE = HS ** -0.5

F32 = mybir.dt.float32
F32R = mybir.dt.float32r
BF16 = mybir.dt.bfloat16
Exp = mybir.ActivationFunctionType.Exp


def build_kernel():
    nc = bacc.Bacc("TRN2", target_bir_lowering=False)

    # all inputs pre-swizzled on host into SBUF layout: straight row-contiguous
    # copies with large DMA descriptors (fast per-queue transfer)
    xt_d = nc.dram_tensor("xt", (P, 4 * 4096), BF16, kind="ExternalInput")
    wqk_d = nc.dram_tensor("wqk", (P, 8 * 1024), BF16, kind="ExternalInput")
    bqk_d = nc.dram_tensor("bqk", (8 * P,), F32, kind="ExternalInput")
    wv_d = nc.dram_tensor("wv", (P, 8 * 512), BF16, kind="ExternalInput")
    bv_d = nc.dram_tensor("bv", (1, 512), F32R, kind="ExternalInput")
    wproj_d = nc.dram_tensor("wproj", (P, 4 * C), BF16, kind="ExternalInput")
    y_d = nc.dram_tensor("y", (T, C), BF16, kind="ExternalOutput")

    with tile.TileContext(nc) as tc, ExitStack() as big:
        const = big.enter_context(tc.tile_pool(name="const", bufs=1))
        persist = big.enter_context(tc.tile_pool(name="persist", bufs=1))
        xtp = big.enter_context(tc.tile_pool(name="xtp", bufs=2))
        atp = big.enter_context(tc.tile_pool(name="atp", bufs=3))
        rzp = big.enter_context(tc.tile_pool(name="rzp", bufs=2))
        rbp = big.enter_context(tc.tile_pool(name="rbp", bufs=2))
        ysp = big.enter_context(tc.tile_pool(name="ysp", bufs=3))
        ps_s = big.enter_context(tc.tile_pool(name="ps_s", bufs=2, space="PSUM"))
        ps_po = big.enter_context(tc.tile_pool(name="ps_po", bufs=1, space="PSUM"))
        ps_acc = big.enter_context(tc.tile_pool(name="ps_acc", bufs=2, space="PSUM"))

        # ---------------- input DMAs first: no waits, big transfers --------
        wqk_sb = persist.tile([P, 8 * 1024], BF16, tag="wqk")
        wv_sb = persist.tile([P, 8 * 512], BF16, tag="wv")
        wpj = persist.tile([P, 4 * C], BF16, tag="wpj")
        bqk = persist.tile([P, 8], F32, tag="bqk")
        bias_v = persist.tile([P, 512], F32, tag="bias_v")
        bvr = const.tile([1, 512], F32R, tag="bvr")
        xtc = [None] * 4

        def load_chunk(tck, ways=2):
            xtc[tck] = xtp.tile([P, 8 * 512], BF16, tag="xT", name=f"xT{tck}")
            rows = P // ways
            for w in range(ways):
                nc.sync.dma_start(
                    xtc[tck][w * rows : (w + 1) * rows, :],
                    xt_d[w * rows : (w + 1) * rows, tck * 4096 : (tck + 1) * 4096],
                )

        # x chunk0 and wv first (v groups can start before wqk finishes)
        load_chunk(0, ways=4)
        for w in range(2):
            nc.sync.dma_start(
                wv_sb[w * 64 : (w + 1) * 64, :], wv_d[w * 64 : (w + 1) * 64, :]
            )
        nc.sync.dma_start(bvr[:], bv_d[:])
        for w in range(4):
            nc.sync.dma_start(
                wqk_sb[w * 32 : (w + 1) * 32, :], wqk_d[w * 32 : (w + 1) * 32, :]
            )
        nc.sync.dma_start(bqk[:], bqk_d[:].rearrange("(a p) -> p a", p=P))

        # ---------------- constants ----------------
        mask = const.tile([P, P], BF16, tag="mask")
        make_upper_triangular(nc, mask[:], val=1.0, diag=True)
        ones_f = const.tile([P, P], F32, tag="ones_f")
        nc.vector.memset(ones_f[:], 1.0)
        ones_t = const.tile([1, P], F32R, tag="ones")
        nc.vector.tensor_copy(ones_t[:], ones_f[0:1, :])
        # selA rows {32p}: cols 0:64 = 1; selB rows {32p}: cols 64:128 = 1
        ones_b = const.tile([P, P], BF16, tag="ones_b")
        nc.vector.memset(ones_b[:], 1.0)
        selA = const.tile([P, P], BF16, tag="selA")
        selB = const.tile([P, P], BF16, tag="selB")
        nc.vector.memset(selA[:], 0.0)
        nc.vector.memset(selB[:], 0.0)
        for pr in range(4):
            nc.sync.dma_start(
                selA[pr * 32 : pr * 32 + 1, 0:64], ones_b[0:1, 0:64]
            )
            nc.sync.dma_start(
                selB[pr * 32 : pr * 32 + 1, 64:P], ones_b[0:1, 0:64]
            )

        # ---------------- persistent tensors ----------------
        # qk_all: per pair p: block 2p = q (rows 0:64 head A, 64:128 head B),
        # block 2p+1 = k (same row split). [P, 8*T] bf16.
        qk_all = persist.tile([P, 8 * T], BF16, tag="qk")
        # v_all: per (pair, kb): [vA(64) | onesA(1) | vB(64) | onesB(1)] = 130
        v_all = persist.tile([P, 4 * NKB * 130], BF16, tag="v")
        va4 = v_all[:].rearrange("p (a b c) -> p a b c", a=4, b=NKB, c=130)
        nc.vector.tensor_copy(va4[:, :, :, 64:65], ones_f[:, 0 : 4 * NKB])
        nc.vector.tensor_copy(va4[:, :, :, 129:130], ones_f[:, 0 : 4 * NKB])
        # unnormalized attention output, pair-stacked transposed layout
        aoT = persist.tile([P, 4 * T], BF16, tag="aoT")
        # bias_v[128, 512] = b_v broadcast along partitions (K=1 matmul)
        pbv = ps_acc.tile([P, 512], F32, tag="acc")
        nc.tensor.matmul(pbv[:], ones_t[:], bvr[:], start=True, stop=True)
        nc.vector.tensor_copy(bias_v[:], pbv[:])

        # ---------------- work-item generators ----------------
        # fill work is split into ~2-matmul micro-items so draining between
        # attention blocks never delays the next scores issue by more than
        # ~450ns (big lumps slip the ACT chain)
        def qk_items(tck, chb):
            st = {}

            def mms(c0):
                def run():
                    if c0 == 0:
                        st["pq"] = ps_acc.tile([P, 512], F32, tag="acc", name="pq")
                    for cb in (c0, c0 + 1):
                        nc.tensor.matmul(
                            st["pq"][:],
                            wqk_sb[:, cb * 1024 + chb * P : cb * 1024 + (chb + 1) * P],
                            xtc[tck][:, cb * 512 : (cb + 1) * 512],
                            start=(cb == 0),
                            stop=(cb == 7),
                        )
                return run

            def evict():
                p_pair, kind = chb // 2, chb % 2  # kind: 0 = q, 1 = k
                blk = 2 * p_pair + kind
                nc.vector.tensor_scalar_add(
                    qk_all[:, blk * T + tck * 512 : blk * T + (tck + 1) * 512],
                    st["pq"][:],
                    bqk[:, chb : chb + 1],
                )

            return [mms(0), mms(2), mms(4), mms(6), evict]

        def qk_group(tck, chb):
            items = qk_items(tck, chb)

            def run():
                for it in items:
                    it()
            return run

        def v_items(tck, tb):
            st = {}
            kb = tck * 4 + tb

            def mms(c0):
                def run():
                    if c0 == 0:
                        st["pv"] = ps_acc.tile([P, 512], F32, tag="acc", name="pv")
                    for cb in (c0, c0 + 1):
                        nc.tensor.matmul(
                            st["pv"][:],
                            xtc[tck][:, cb * 512 + tb * P : cb * 512 + (tb + 1) * P],
                            wv_sb[:, cb * 512 : (cb + 1) * 512],
                            start=(cb == 0),
                            stop=(cb == 7),
                        )
                return run

            def evict():
                pv = st["pv"]
                dst = bass.AP(
                    v_all[:].tensor,
                    v_all[:].offset + kb * 130,
                    [[v_all[:].ap[0][0], P], [NKB * 130, 4], [65, 2], [1, 64]],
                )
                src = bass.AP(
                    pv[:].tensor,
                    pv[:].offset,
                    [[pv[:].ap[0][0], P], [128, 4], [64, 2], [1, 64]],
                )
                bsrc = bass.AP(
                    bias_v[:].tensor,
                    bias_v[:].offset,
                    [[bias_v[:].ap[0][0], P], [128, 4], [64, 2], [1, 64]],
                )
                nc.vector.tensor_tensor(dst, src, bsrc, mybir.AluOpType.add)

            return [mms(0), mms(2), mms(4), mms(6), evict]

        def v_group(tck, tb):
            items = v_items(tck, tb)

            def run():
                for it in items:
                    it()
            return run

        def chunk_items(tck):
            items = []
            for chb in range(8):
                items += qk_items(tck, chb)
            for tb in range(4):
                items += v_items(tck, tb)
            return items

        def proj_micro(qc, tb, oc):
            st = {}

            def mms(p0):
                def run():
                    if p0 == 0:
                        st["py"] = ps_acc.tile([P, 512], F32, tag="acc", name="py")
                    for pp in (p0, p0 + 1):
                        nc.tensor.matmul(
                            st["py"][:],
                            aoT[:, pp * T + tb * P : pp * T + (tb + 1) * P],
                            wpj[:, pp * C + oc * 512 : pp * C + (oc + 1) * 512],
                            start=(pp == 0),
                            stop=(pp == 3),
                        )
                return run

            def evict():
                ys = ysp.tile([P, 512], BF16, tag="ys")
                nc.vector.tensor_copy(ys[:], st["py"][:])
                nc.sync.dma_start(
                    y_d[tb * P : (tb + 1) * P, oc * 512 : (oc + 1) * 512], ys[:]
                )

            return [mms(0), mms(2), evict]

        def proj_items(qc):
            items = []
            for tb in range(qc * 4, (qc + 1) * 4):
                for oc in range(2):
                    items += proj_micro(qc, tb, oc)
            return items

        def norm_pair(qc, rz, p_pair):
            # rz row 32p holds [Z_A (512) | Z_B (512)] for pair p. Two
            # accumulating K=1 broadcast matmuls -> pbt (rows 0:64 = Z_A,
            # 64:128 = Z_B), reciprocal after broadcast, one [128,512] mul.
            r = p_pair * 32
            pbt = ps_acc.tile([P, 512], F32, tag="acc")
            nc.tensor.matmul(
                pbt[:],
                selA[r : r + 1, :],
                rz[r : r + 1, 0:512],
                start=True,
                stop=False,
                tile_position=(r, 0),
            )
            nc.tensor.matmul(
                pbt[:],
                selB[r : r + 1, :],
                rz[r : r + 1, 512:1024],
                start=False,
                stop=True,
                tile_position=(r, 0),
            )
            rb = rbp.tile([P, 512], F32, tag="rb")
            nc.vector.reciprocal_approx_fast(rb[:], pbt[:])
            col = p_pair * T + qc * 512
            nc.vector.tensor_mul(
                aoT[:, col : col + 512], aoT[:, col : col + 512], rb[:]
            )

        # ---------------- attention rounds with static interleave ----------
        fill_q = deque()

        def drain(n):
            for _ in range(n):
                if fill_q:
                    fill_q.popleft()()

        def run_round(qc, rz, prereq=None, late_prereq=None):
            nkb = 4 * qc + 4
            n_groups_total = 4 * nkb
            fill_total = len(fill_q)
            done_fill = [0]
            done_groups = [0]

            def after_group():
                done_groups[0] += 1
                want = fill_total * done_groups[0] // n_groups_total
                d = want - done_fill[0]
                done_fill[0] += d
                drain(d)

            pending_norm = [None]
            for p_pair in range(4):
                if prereq is not None:
                    for it in prereq(p_pair):
                        it()
                qblk, kblk = 2 * p_pair, 2 * p_pair + 1
                po0 = ps_po.tile([65, 512], F32, tag="po0")
                po1 = ps_po.tile([65, 512], F32, tag="po1")
                po = [po0, po1]
                pend = None  # (at, kb)
                # diagonal blocks first: their masks/GpSimd work lands while
                # the vector queue is free, and the pair tail stays clean
                kb_order = list(range(4 * qc, nkb)) + list(range(0, 4 * qc))
                first_kb, last_kb = kb_order[0], kb_order[-1]
                for kb in kb_order:
                    if late_prereq is not None:
                        for it in late_prereq(p_pair, kb):
                            it()
                    qoff = max(0, kb * P - qc * 512)
                    S = ps_s.tile([P, 1024], F32, tag="S")
                    at = atp.tile([P, 1024], BF16, tag="at")
                    for hh in range(2):
                        r0 = hh * 64
                        nc.tensor.matmul(
                            S[:, hh * 512 + qoff : (hh + 1) * 512],
                            qk_all[r0 : r0 + 64, kblk * T + kb * P : kblk * T + (kb + 1) * P],
                            qk_all[r0 : r0 + 64, qblk * T + qc * 512 + qoff : qblk * T + (qc + 1) * 512],
                            start=True,
                            stop=True,
                        )
                    if qoff >= 256:
                        # mostly-garbage diagonal tile: exp only valid spans;
                        # shortens the ACTs that pace each pair's start
                        for hh in range(2):
                            c0 = hh * 512 + qoff
                            nc.scalar.activation(
                                at[:, c0 : (hh + 1) * 512], S[:, c0 : (hh + 1) * 512],
                                Exp, scale=SCALE,
                            )
                    else:
                        nc.scalar.activation(at[:], S[:], Exp, scale=SCALE)
                    if kb * P >= qc * 512:  # diagonal: zero k > q entries
                        # diag blocks run first in each pair, so these hit the
                        # Vector queue while it is free (evictions are at pair
                        # ends) — and DVE is ~3x faster than GpSimd here
                        for hh in range(2):
                            c0 = hh * 512 + qoff
                            nc.vector.tensor_mul(
                                at[:, c0 : c0 + P], at[:, c0 : c0 + P], mask[:]
                            )
                    if pend is not None:
                        emit_attnv(qc, p_pair, po, *pend, first_kb, last_kb)
                    pend = (at, kb)
                    after_group()
                    if pending_norm[0] is not None:
                        # previous pair's normalization, off the critical path
                        norm_pair(qc, rz, pending_norm[0])
                        pending_norm[0] = None
                emit_attnv(qc, p_pair, po, *pend, first_kb, last_kb)
                # evict raw ao split across ScalarE and VectorE + Z rows
                col = p_pair * T + qc * 512
                nc.scalar.copy(aoT[0:64, col : col + 512], po0[0:64, :])
                nc.vector.tensor_copy(aoT[64:P, col : col + 512], po1[0:64, :])
                r = p_pair * 32
                nc.vector.tensor_copy(rz[r : r + 1, 0:512], po0[64:65, :])
                nc.vector.tensor_copy(rz[r : r + 1, 512:1024], po1[64:65, :])
                pending_norm[0] = p_pair
            drain(len(fill_q))
            norm_pair(qc, rz, pending_norm[0])

        def emit_attnv(qc, p_pair, po, at, kb, first_kb, last_kb):
            qoff = max(0, kb * P - qc * 512)
            for hh in range(2):
                nc.tensor.matmul(
                    po[hh][:, qoff:512],
                    v_all[:, p_pair * NKB * 130 + kb * 130 + hh * 65 :
                          p_pair * NKB * 130 + kb * 130 + hh * 65 + 65],
                    at[:, hh * 512 + qoff : (hh + 1) * 512],
                    start=(kb == first_kb),
                    stop=(kb == last_kb),
                    skip_group_check=True,
                )

        # ---------------- main schedule ----------------
        # round 0 pair p only needs its own q/k groups (chb 2p, 2p+1) and the
        # first four v groups; emit the minimum up front, rest per-pair.
        for it in [v_group(0, tb) for tb in range(4)] + [qk_group(0, 0), qk_group(0, 1)]:
            it()

        def prereq0(p_pair):
            if p_pair == 0:
                return []
            return [qk_group(0, 2 * p_pair), qk_group(0, 2 * p_pair + 1)]

        # round 3's k and v chunk-3 groups are deferred INTO round 3 (they are
        # only needed from kb 12 onward) to fill its ACT-paced stalls.
        def late3(p_pair, kb):
            items = []
            if kb == 12:
                items.append(qk_group(3, 2 * p_pair + 1))
                if p_pair == 0:
                    items.append(v_group(3, 0))
            elif p_pair == 0 and kb in (13, 14, 15):
                items.append(v_group(3, kb - 12))
            return items

        rzs = [None] * 4
        for qc in range(4):
            rzs[qc] = rzp.tile([P, 1024], BF16, tag="rz", name=f"rz{qc}")
            if qc < 3:
                fill_q.append(lambda t=qc + 1: load_chunk(t))
            if qc == 0:
                def load_wpj():
                    for w in range(2):
                        nc.sync.dma_start(
                            wpj[w * 64 : (w + 1) * 64, :],
                            wproj_d[w * 64 : (w + 1) * 64, :],
                        )
                fill_q.append(load_wpj)
            if qc < 2:
                fill_q.extend(chunk_items(qc + 1))
            elif qc == 2:
                # only chunk 3's q groups (k/v deferred into round 3)
                fill_q.extend(qk_group(3, 2 * p) for p in range(4))
                fill_q.extend(proj_items(0))
                fill_q.extend(proj_items(1))
            if qc == 3:
                fill_q.extend(proj_items(2))
            run_round(
                qc,
                rzs[qc],
                prereq=prereq0 if qc == 0 else None,
                late_prereq=late3 if qc == 3 else None,
            )
        for it in proj_items(3):
            it()

    nc.compile()
    return nc


def _shard_inputs(x, W_qkv, b_qkv, W_proj):
    """Build the 8 per-core input maps."""
    in_maps = []
    for c in range(8):
        b = c // 2
        hg = c % 2
        heads = [hg * 8 + j for j in range(8)]
        qk_cols = []
        for p in range(4):
            ha, hb = heads[2 * p], heads[2 * p + 1]
            for part in range(2):  # q, k
                qk_cols.extend(range(ha * 192 + part * 64, ha * 192 + part * 64 + 64))
                qk_cols.extend(range(hb * 192 + part * 64, hb * 192 + part * 64 + 64))
        qk_cols = np.array(qk_cols)
        v_cols = []
        for p in range(4):
            ha, hb = heads[2 * p], heads[2 * p + 1]
            v_cols.extend(range(ha * 192 + 128, ha * 192 + 192))
            v_cols.extend(range(hb * 192 + 128, hb * 192 + 192))
        v_cols = np.array(v_cols)
        # pre-swizzle into the exact SBUF layouts (row-contiguous for fast DMA)
        # xt[p, tck*4096 + cb*512 + t'] = x[b][tck*512 + t', cb*128 + p]
        xt = (
            x[b].reshape(4, 512, 8, 128).transpose(3, 0, 2, 1).reshape(128, 16384)
        )
        # wqk[p, cb*1024 + ch] = W_qkv[cb*128 + p, qk_cols[ch]]
        wqk = (
            W_qkv[:, qk_cols].reshape(8, 128, 1024).transpose(1, 0, 2).reshape(128, 8192)
        )
        wv = (
            W_qkv[:, v_cols].reshape(8, 128, 512).transpose(1, 0, 2).reshape(128, 4096)
        )
        wproj = (
            W_proj[hg * 512 : (hg + 1) * 512, :]
            .reshape(4, 128, 1024).transpose(1, 0, 2).reshape(128, 4096)
        )
        in_maps.append(
            {
                "xt": np.ascontiguousarray(xt.astype(ml_dtypes.bfloat16)),
                "wqk": np.ascontiguousarray(wqk.astype(ml_dtypes.bfloat16)),
                "bqk": np.ascontiguousarray(b_qkv[qk_cols], dtype=np.float32),
                "wv": np.ascontiguousarray(wv.astype(ml_dtypes.bfloat16)),
                "bv": np.ascontiguousarray(
                    b_qkv[v_cols].reshape(1, 512), dtype=np.float32
                ),
                "wproj": np.ascontiguousarray(wproj.astype(ml_dtypes.bfloat16)),
            }
        )
    return in_maps


_NC = None


def kernel(x, W_qkv, b_qkv, W_proj, b_proj, _trace=False):
    global _NC
    x = np.asarray(x, dtype=np.float32)
    W_qkv = np.asarray(W_qkv, dtype=np.float32)
    b_qkv = np.asarray(b_qkv, dtype=np.float32)
    W_proj = np.asarray(W_proj, dtype=np.float32)
    b_proj = np.asarray(b_proj, dtype=np.float32)

    in_maps = _shard_inputs(x, W_qkv, b_qkv, W_proj)
    if _NC is None:
        _NC = build_kernel()
    res = run_bass_kernel_spmd(
        _NC, in_maps, core_ids=list(range(8)), trace=_trace,
        trace_cores=list(range(8)) if _trace else None,
    )
    out = np.empty((B, T, C), dtype=np.float32)
    for b in range(B):
        out[b] = (
            res.results[2 * b]["y"].astype(np.float32)
            + res.results[2 * b + 1]["y"].astype(np.float32)
            + b_proj
        )
    if _trace:
        return out, res
    return out
